# revision 26
# baseline (speedup 1.0000x reference)
"""GCN link predictor on 8 Trainium2 NeuronCores (Bass/Tile).

Math (reference up to fp reassociation + bf16 feature rounding):
    dinv = deg^-1/2                    (host)
    x'   = bf16(dinv * x)              (host prescale, sharded by node)
    S    = sum_e dinv[col] * x'[row]   (device: gather + dma_scatter_add)
    h1   = bf16(dinv * relu(S @ W1 + b1))
    S2   = aggregate(h1);  h2 = bf16(S2 @ W2 + b2)
    out  = relu([h2[s], h2[d]] @ Wm1 + bm1) @ Wm2 + bm2

Sharding: destinations are range-sharded (12500 nodes/core); features are
exchanged with bf16 AllGathers (3.2 MB/rank). The segment sum runs on the
DMA engines via dma_scatter_add: messages are grouped into "rounds" (rank
within destination), so every scatter call touches each address at most
once — race-free — and the tile framework's WAW deps serialize rounds.
Accumulation is fp32; features/weights are bf16 (validated ~2e-3 rel err).

kernel() compiles once, stages inputs on device, then reports the
steady-state per-run execution wall time over a pipelined batch of runs
(LAST_RUN_S); LAST_SINGLE_S is a single blocking dispatch+execute.
"""
import sys
import time
import numpy as np

sys.path.insert(0, "/opt/trn_rl_repo")

LAST_RUN_S = None      # steady-state per-run wall time (pipelined batch)
LAST_SINGLE_S = None   # single blocking dispatch+execute wall time

# ---------------- configuration (full problem; hardcoded) ----------------
N_NODES = 100000
C = 128
N_PAIRS = 500000
NCORES = 8
SLICE = N_NODES // NCORES            # 12500 nodes per core
ROWS = ((SLICE + 127) // 128) * 128  # 12544 padded rows per core
NTILE = ROWS // 128                  # 98
GROWS = NCORES * ROWS                # 100352
NW = 4
WIN = GROWS // NW                    # 25088 (< 32768 so int16 works)
DUMP = ROWS                          # dump row for pad messages
SROWS = ROWS + 128                   # 12672 (dump block included)
PAIR_BLK = 512                       # pairs per MLP block (one PSUM bank)
CALL_MAX = 4096                      # msgs per dma_gather/scatter call
                                     # (ring fits with single_packet=False)
TW = 4                               # dest tiles per transform group
N_TIMED = 32                         # pipelined timed iterations


def _gmap(n):
    return (n // SLICE) * ROWS + (n % SLICE)


def _wrap16(stream):
    """[K] int -> [16, K/16] int16 wrapped layout (replicated to 128
    partitions on device)."""
    return np.ascontiguousarray(
        np.asarray(stream, np.int16).reshape(-1, 16).T)


def _build_message_streams(row, col, dinv):
    """Per-core gather/scatter streams in (round, window) order.

    A message's round is its rank within its destination; within a round
    destinations are unique, so one dma_scatter_add per (piece of a) round
    is race-free. Segments (round, window) are padded to a shared
    (max-over-cores) multiple of 128 so the program is SPMD-uniform.
    Pad slots: gather idx 0, dmsg 0.0 (payload zeroed), scatter idx DUMP.

    Returns gidx/sidx [NCORES, 16, tot/16] i16, dmsg [NCORES, 128, tot/128]
    bf16-able f32, and rounds = list of [(window, seg_len), ...].
    """
    E = len(row)
    core = col // SLICE
    src_pos = _gmap(row)
    win = src_pos // WIN
    widx = src_pos % WIN
    dloc = col % SLICE

    order_by_dest = np.argsort(col, kind="stable")
    sc = col[order_by_dest]
    run_start = np.r_[0, np.flatnonzero(np.diff(sc)) + 1]
    run_len = np.diff(np.r_[run_start, E])
    rank_sorted = np.arange(E) - np.repeat(run_start, run_len)
    rank = np.empty(E, np.int64)
    rank[order_by_dest] = rank_sorted
    R = int(rank.max()) + 1
    assert R < 127, R

    key = (core * R + rank) * NW + win
    counts = np.bincount(key, minlength=NCORES * R * NW).reshape(
        NCORES, R, NW)
    Lseg = ((counts.max(axis=0) + 127) // 128) * 128
    Lseg[counts.max(axis=0) == 0] = 0

    seg_base = np.zeros((R, NW), np.int64)
    rounds = []
    pos = 0
    for r in range(R):
        segs = []
        for w in range(NW):
            L = int(Lseg[r, w])
            if L == 0:
                continue
            seg_base[r, w] = pos
            segs.append((w, L))
            pos += L
        if segs:
            rounds.append(segs)
    tot = pos

    gidx = np.zeros((NCORES, tot), np.int64)
    sidx = np.full((NCORES, tot), DUMP, np.int64)
    dmsg = np.zeros((NCORES, tot), np.float32)
    order = np.argsort(key, kind="stable")
    core_s = core[order]
    flat_base = seg_base.reshape(-1)
    for k in range(NCORES):
        sel = order[core_s == k]
        ck = counts[k].reshape(-1)
        seg_starts = np.r_[0, np.cumsum(ck)[:-1]]
        off = np.arange(len(sel)) - np.repeat(seg_starts, ck)
        base = flat_base[np.repeat(np.arange(R * NW), ck)]
        p = base + off
        gidx[k, p] = widx[sel]
        sidx[k, p] = dloc[sel]
        dmsg[k, p] = dinv[col[sel]]
    return gidx, sidx, dmsg, rounds, tot


def _build_pair_streams(edge_pairs):
    """Per-core src/dst pair-gather streams grouped by (src win, dst win)."""
    pp = N_PAIRS // NCORES
    src = _gmap(edge_pairs[0])
    dst = _gmap(edge_pairs[1])
    g = (src // WIN) * NW + (dst // WIN)
    counts = np.zeros((NCORES, NW * NW), np.int64)
    for k in range(NCORES):
        counts[k] = np.bincount(g[k * pp:(k + 1) * pp], minlength=NW * NW)
    Lg = ((counts.max(axis=0) + PAIR_BLK - 1) // PAIR_BLK) * PAIR_BLK
    Lg[counts.max(axis=0) == 0] = 0
    group_plan = [(gi // NW, gi % NW, int(Lg[gi]))
                  for gi in range(NW * NW) if Lg[gi] > 0]
    base = np.r_[0, np.cumsum(Lg)][:-1]
    totp = int(Lg.sum())

    psrc = np.zeros((NCORES, totp), np.int64)
    pdst = np.zeros((NCORES, totp), np.int64)
    omap = np.full((NCORES, totp), -1, np.int64)
    for k in range(NCORES):
        gk = g[k * pp:(k + 1) * pp]
        order = np.argsort(gk, kind="stable")
        ck = counts[k]
        seg_starts = np.r_[0, np.cumsum(ck)[:-1]]
        off = np.arange(pp) - np.repeat(seg_starts, ck)
        p = base[gk[order]] + off
        psrc[k, p] = src[k * pp:(k + 1) * pp][order] % WIN
        pdst[k, p] = dst[k * pp:(k + 1) * pp][order] % WIN
        omap[k, p] = order
    return psrc, pdst, omap, group_plan, totp


# ---------------- device kernel builder ----------------

def _build(nc, rounds, tot, group_plan, totp, stage=None):
    """stage: None/'full' = whole program; 'ag' = setup+AllGather only;
    'round0' = + zero S + first scatter round; 'msg1' = + full layer-1
    message phase; 'l1' = + transform1 + AllGather h1. Debug stages write
    a dbg [256, C] f32 output instead of the scoring output."""
    import concourse.bass as bass
    import concourse.mybir as mybir
    from concourse import tile

    f32 = mybir.dt.float32
    b16 = mybir.dt.bfloat16
    i16 = mybir.dt.int16
    AF = mybir.ActivationFunctionType
    MUL = mybir.AluOpType.mult
    ADD = mybir.AluOpType.add
    T16 = tot // 16
    T128 = tot // 128
    P16 = totp // 16

    xl = nc.dram_tensor("xl", [ROWS, C], b16, kind="ExternalInput")
    dinv_t = nc.dram_tensor("dinv_t", [128, NTILE], f32,
                            kind="ExternalInput")
    ident = nc.dram_tensor("ident", [C, C], b16, kind="ExternalInput")
    g16 = nc.dram_tensor("g16", [16, T16], i16, kind="ExternalInput")
    s16 = nc.dram_tensor("s16", [16, T16], i16, kind="ExternalInput")
    dmsg = nc.dram_tensor("dmsg", [128, T128], b16, kind="ExternalInput")
    p16s = nc.dram_tensor("p16s", [16, P16], i16, kind="ExternalInput")
    p16d = nc.dram_tensor("p16d", [16, P16], i16, kind="ExternalInput")
    w1 = nc.dram_tensor("w1", [C, C], b16, kind="ExternalInput")
    w2 = nc.dram_tensor("w2", [C, C], b16, kind="ExternalInput")
    wm1a = nc.dram_tensor("wm1a", [C, C], b16, kind="ExternalInput")
    wm1b = nc.dram_tensor("wm1b", [C, C], b16, kind="ExternalInput")
    wm2 = nc.dram_tensor("wm2", [C, 1], b16, kind="ExternalInput")
    b1r = nc.dram_tensor("b1r", [1, C], f32, kind="ExternalInput")
    b2r = nc.dram_tensor("b2r", [1, C], f32, kind="ExternalInput")
    bm1c = nc.dram_tensor("bm1c", [C, 1], f32, kind="ExternalInput")
    bm2c = nc.dram_tensor("bm2c", [1, 1], f32, kind="ExternalInput")

    if stage in (None, "full"):
        out = nc.dram_tensor("out", [totp], f32, kind="ExternalOutput")
    else:
        dbg = nc.dram_tensor("dbg", [256, C], f32, kind="ExternalOutput")

    xli = nc.dram_tensor("xli", [ROWS, C], b16)
    xp = nc.dram_tensor("xp", [GROWS, C], b16, addr_space="Shared")
    h1l = nc.dram_tensor("h1l", [ROWS, C], b16)
    h1p = nc.dram_tensor("h1p", [GROWS, C], b16, addr_space="Shared")
    h2l = nc.dram_tensor("h2l", [ROWS, C], b16)
    h2p = nc.dram_tensor("h2p", [GROWS, C], b16, addr_space="Shared")
    # two accumulators, rounds alternate: halves the scatter WAW chain.
    # bf16 accumulation halves scatter RMW bytes (validated rel ~2.5e-3)
    S0 = nc.dram_tensor("S0", [SROWS, C], b16)
    S1 = nc.dram_tensor("S1", [SROWS, C], b16)

    replica = [list(range(NCORES))]
    ZB = 20  # zero-write blocks per DMA

    def msg_layer(pools, src_buf, gsb, ssb, dmsb, zt, max_rounds=None):
        pbp, pfp = pools
        # zero both accumulators (dump block included)
        nb = SROWS // 128  # 99
        for Sx in (S0, S1):
            z0 = 0
            while z0 < nb:
                zw = min(ZB, nb - z0)
                nc.sync.dma_start(
                    Sx[z0 * 128:(z0 + zw) * 128, :].rearrange(
                        "(b p) c -> p b c", p=128),
                    zt[:, :zw * C].rearrange("p (b c) -> p b c", c=C))
                z0 += zw
        off = 0
        qctr = [0]
        rl = rounds if max_rounds is None else rounds[:max_rounds]
        for ri, segs in enumerate(rl):
            Sx = S0 if ri % 2 == 0 else S1
            # split the round into <=2 contiguous pieces to bound SBUF
            mid = (len(segs) + 1) // 2
            for half in (segs[:mid], segs[mid:]):
                if not half:
                    continue
                Lh = sum(L for (_w, L) in half)
                pb = pbp.tile([128, Lh // 128, C], b16, tag="pb")
                o = 0
                for (w, L) in half:
                    src_ap = src_buf[w * WIN:(w + 1) * WIN, :]
                    for g0 in range(0, L, CALL_MAX):
                        gl = min(CALL_MAX, L - g0)
                        oo = off + o + g0
                        nc.gpsimd.dma_gather(
                            pb[:, (o + g0) // 128:(o + g0 + gl) // 128, :],
                            src_ap, gsb[:, oo // 16:(oo + gl) // 16],
                            gl, gl, C, elem_step=C,
                            queue_num=qctr[0] % 2,
                            single_packet=False)
                        qctr[0] += 1
                    o += L
                pf = pfp.tile([128, Lh // 128, C], b16, tag="pf")
                dm = dmsb[:, off // 128:(off + Lh) // 128]
                d3 = bass.AP(dm.tensor, dm.offset,
                             [dm.ap[0], dm.ap[1], [0, C]])
                nc.vector.tensor_tensor(pf[:, :, :], pb[:, :, :], d3, op=MUL)
                for g0 in range(0, Lh, CALL_MAX):
                    gl = min(CALL_MAX, Lh - g0)
                    oo = off + g0
                    nc.gpsimd.dma_scatter_add(
                        Sx[:, :], pf[:, g0 // 128:(g0 + gl) // 128, :],
                        ssb[:, oo // 16:(oo + gl) // 16],
                        gl, gl, C, queue_num=2 + ri % 2,
                        single_packet=False)
                off += Lh

    def transform(pools, w_sb, b_bc, dinv_sb, ident_sb, hout, relu_dinv):
        tp, psT, ps2 = pools
        t0 = 0
        while t0 < NTILE:
            tw = min(TW, NTILE - t0)
            r0, r1 = t0 * 128, (t0 + tw) * 128
            s4 = tp.tile([128, tw, C], b16, tag="s4")
            nc.sync.dma_start(
                s4[:, :, :],
                S0[r0:r1, :].rearrange("(b p) c -> p b c", p=128))
            s4b = tp.tile([128, tw, C], b16, tag="s4b")
            nc.sync.dma_start(
                s4b[:, :, :],
                S1[r0:r1, :].rearrange("(b p) c -> p b c", p=128))
            nc.vector.tensor_tensor(s4[:, :, :], s4[:, :, :], s4b[:, :, :],
                                    op=ADD)
            pt = psT.tile([128, TW, C], b16, tag="pt")
            for b in range(tw):
                nc.tensor.transpose(pt[:, b, :], s4[:, b, :], ident_sb)
            stt = tp.tile([128, tw, C], b16, tag="stt")
            nc.scalar.copy(stt[:, :, :], pt[:, :tw, :])
            pg = ps2.tile([128, TW, C], f32, tag="pg")
            for b in range(tw):
                nc.tensor.matmul(pg[:, b, :], stt[:, b, :], w_sb,
                                 start=True, stop=True)
            zf = tp.tile([128, tw, C], f32, tag="zf")
            b3 = bass.AP(b_bc.tensor, b_bc.offset,
                         [b_bc.ap[0], [0, tw], b_bc.ap[1]])
            nc.vector.tensor_tensor(zf[:, :, :], pg[:, :tw, :], b3, op=ADD)
            h4 = tp.tile([128, tw, C], b16, tag="h4")
            if relu_dinv:
                for b in range(tw):
                    nc.scalar.activation(
                        h4[:, b, :], zf[:, b, :], AF.Relu,
                        scale=dinv_sb[:, t0 + b:t0 + b + 1])
            else:
                nc.scalar.copy(h4[:, :, :], zf[:, :, :])
            nc.sync.dma_start(
                hout[r0:r1, :].rearrange("(b p) c -> p b c", p=128),
                h4[:, :, :])
            t0 += tw

    with tile.TileContext(nc) as tc:
        with tc.tile_pool(name="cst", bufs=1) as cst:
            w1_sb = cst.tile([C, C], b16)
            nc.sync.dma_start(w1_sb[:, :], w1[:, :])
            w2_sb = cst.tile([C, C], b16)
            nc.sync.dma_start(w2_sb[:, :], w2[:, :])
            wm1a_sb = cst.tile([C, C], b16)
            nc.sync.dma_start(wm1a_sb[:, :], wm1a[:, :])
            wm1b_sb = cst.tile([C, C], b16)
            nc.sync.dma_start(wm1b_sb[:, :], wm1b[:, :])
            wm2_sb = cst.tile([C, 1], b16)
            nc.sync.dma_start(wm2_sb[:, :], wm2[:, :])
            bm1_sb = cst.tile([C, 1], f32)
            nc.sync.dma_start(bm1_sb[:, :], bm1c[:, :])
            bm2_sb = cst.tile([1, 1], f32)
            nc.sync.dma_start(bm2_sb[:, :], bm2c[:, :])
            dinv_sb = cst.tile([128, NTILE], f32)
            nc.sync.dma_start(dinv_sb[:, :], dinv_t[:, :])
            ident_sb = cst.tile([C, C], b16)
            nc.sync.dma_start(ident_sb[:, :], ident[:, :])
            b1_row = cst.tile([1, C], f32)
            nc.sync.dma_start(b1_row[:, :], b1r[:, :])
            b2_row = cst.tile([1, C], f32)
            nc.sync.dma_start(b2_row[:, :], b2r[:, :])
            ones_sb = cst.tile([1, C], f32)
            nc.vector.memset(ones_sb[:, :], 1.0)
            # partition-broadcast biases: b_bc[p, c] = b[c]
            b1_bc = cst.tile([128, C], f32)
            b2_bc = cst.tile([128, C], f32)
            with tc.tile_pool(name="psb", bufs=2, space="PSUM") as psb:
                pb1 = psb.tile([128, C], f32, tag="pb1")
                nc.tensor.matmul(pb1[:, :], ones_sb[:, :], b1_row[:, :],
                                 start=True, stop=True)
                nc.scalar.copy(b1_bc[:, :], pb1[:, :])
                pb2 = psb.tile([128, C], f32, tag="pb2")
                nc.tensor.matmul(pb2[:, :], ones_sb[:, :], b2_row[:, :],
                                 start=True, stop=True)
                nc.scalar.copy(b2_bc[:, :], pb2[:, :])
            # message index streams, replicated x8 down the partitions
            gsb = cst.tile([128, T16], i16)
            ssb = cst.tile([128, T16], i16)
            for i in range(8):
                nc.sync.dma_start(gsb[16 * i:16 * (i + 1), :], g16[:, :])
                nc.sync.dma_start(ssb[16 * i:16 * (i + 1), :], s16[:, :])
            dmsb = cst.tile([128, T128], b16)
            nc.sync.dma_start(dmsb[:, :], dmsg[:, :])
            zt = cst.tile([128, ZB * C], b16)
            nc.vector.memset(zt[:, :], 0.0)

            # collectives may not read IO tensors: stage input -> internal
            nc.sync.dma_start(xli[:, :], xl[:, :])
            nc.gpsimd.collective_compute(
                "AllGather", mybir.AluOpType.bypass, replica_groups=replica,
                ins=[xli.ap().opt()], outs=[xp.ap().opt()])

            def dump(src, src_dtype):
                with tc.tile_pool(name="dmp", bufs=1) as dmp:
                    dt_ = dmp.tile([128, 2, C], src_dtype)
                    nc.sync.dma_start(
                        dt_[:, :, :],
                        src[0:256, :].rearrange("(b p) c -> p b c", p=128))
                    df = dmp.tile([128, 2, C], f32)
                    nc.scalar.copy(df[:, :, :], dt_[:, :, :])
                    nc.sync.dma_start(
                        dbg[0:256, :].rearrange("(b p) c -> p b c", p=128),
                        df[:, :, :])

            if stage == "ag":
                dump(xp, b16)
                return nc

            with (
                tc.tile_pool(name="pbp", bufs=3) as pbp,
                tc.tile_pool(name="pfp", bufs=3) as pfp,
                tc.tile_pool(name="tp", bufs=2) as tp,
                tc.tile_pool(name="psT", bufs=2, space="PSUM") as psT,
                tc.tile_pool(name="ps2", bufs=2, space="PSUM") as ps2,
            ):
                msg_pools = (pbp, pfp)
                t_pools = (tp, psT, ps2)
                mr = 1 if stage == "round0" else None
                msg_layer(msg_pools, xp, gsb, ssb, dmsb, zt, max_rounds=mr)
                if stage in ("round0", "msg1"):
                    dump(S0, b16)  # note: even rounds only with 2 buffers
                    return nc
                transform(t_pools, w1_sb, b1_bc, dinv_sb, ident_sb, h1l,
                          relu_dinv=True)
                nc.gpsimd.collective_compute(
                    "AllGather", mybir.AluOpType.bypass,
                    replica_groups=replica,
                    ins=[h1l.ap().opt()], outs=[h1p.ap().opt()])
                if stage == "l1":
                    dump(h1p, b16)
                    return nc
                msg_layer(msg_pools, h1p, gsb, ssb, dmsb, zt)
                transform(t_pools, w2_sb, b2_bc, dinv_sb, ident_sb, h2l,
                          relu_dinv=False)
                nc.gpsimd.collective_compute(
                    "AllGather", mybir.AluOpType.bypass,
                    replica_groups=replica,
                    ins=[h2l.ap().opt()], outs=[h2p.ap().opt()])

            # ---- scoring MLP over edge pairs ----
            with (
                tc.tile_pool(name="pip", bufs=1) as pip,
                tc.tile_pool(name="pgp", bufs=2) as pgp,
                tc.tile_pool(name="mp", bufs=2) as mp,
                tc.tile_pool(name="psz", bufs=2, space="PSUM") as psz,
                tc.tile_pool(name="pso", bufs=2, space="PSUM") as pso,
            ):
                psb_ = pip.tile([128, P16], i16)
                pdb_ = pip.tile([128, P16], i16)
                for i in range(8):
                    nc.sync.dma_start(psb_[16 * i:16 * (i + 1), :],
                                      p16s[:, :])
                    nc.sync.dma_start(pdb_[16 * i:16 * (i + 1), :],
                                      p16d[:, :])
                opos = 0
                for (sw, dw, Lg) in group_plan:
                    gs = pgp.tile([128, 1, Lg], b16, tag="gs")
                    gd = pgp.tile([128, 1, Lg], b16, tag="gd")
                    TCALL = 512  # transposed-gather call cap
                    for g0 in range(0, Lg, TCALL):
                        gl = min(TCALL, Lg - g0)
                        oo = opos + g0
                        nc.gpsimd.dma_gather(
                            gs[:, :, g0:g0 + gl],
                            h2p[sw * WIN:(sw + 1) * WIN, :],
                            psb_[:, oo // 16:(oo + gl) // 16],
                            gl, gl, C, elem_step=C, transpose=True,
                            queue_num=0)
                        nc.gpsimd.dma_gather(
                            gd[:, :, g0:g0 + gl],
                            h2p[dw * WIN:(dw + 1) * WIN, :],
                            pdb_[:, oo // 16:(oo + gl) // 16],
                            gl, gl, C, elem_step=C, transpose=True,
                            queue_num=1)
                    for b0 in range(0, Lg, PAIR_BLK):
                        pz = psz.tile([128, PAIR_BLK], f32, tag="pz")
                        nc.tensor.matmul(pz[:, :], wm1a_sb[:, :],
                                         gs[:, 0, b0:b0 + PAIR_BLK],
                                         start=True, stop=False)
                        nc.tensor.matmul(pz[:, :], wm1b_sb[:, :],
                                         gd[:, 0, b0:b0 + PAIR_BLK],
                                         start=False, stop=True)
                        z = mp.tile([128, PAIR_BLK], b16, tag="z")
                        nc.scalar.activation(z[:, :], pz[:, :], AF.Relu,
                                             bias=bm1_sb[:, 0:1])
                        po = pso.tile([1, PAIR_BLK], f32, tag="po")
                        nc.tensor.matmul(po[:, :], wm2_sb[:, :], z[:, :],
                                         start=True, stop=True)
                        o = mp.tile([1, PAIR_BLK], f32, tag="o")
                        nc.scalar.activation(o[:, :], po[:, :], AF.Identity,
                                             bias=bm2_sb[:, 0:1])
                        nc.sync.dma_start(out[opos + b0:opos + b0 + PAIR_BLK],
                                          o[0:1, :])
                    opos += Lg
    return nc


# ---------------- SPMD runner (compile once, timed pipelined runs) -------

def _run_spmd(nc, in_maps, n_timed=N_TIMED):
    """Compile nc once, stage inputs on the 8 cores, run 1 warmup + 1
    blocking run + a pipelined batch of n_timed runs. Returns
    (list of per-core output dicts from the last run, single_s, per_run_s).
    """
    import jax
    from jax.sharding import Mesh, PartitionSpec, NamedSharding
    from jax.experimental.shard_map import shard_map
    import concourse.mybir as mybir
    from concourse.bass2jax import (_bass_exec_p, partition_id_tensor,
                                    install_neuronx_cc_hook)

    install_neuronx_cc_hook()
    partition_name = (nc.partition_id_tensor.name
                      if nc.partition_id_tensor else None)
    in_names, out_names, out_avals, zero_outs = [], [], [], []
    for alloc in nc.m.functions[0].allocations:
        if not isinstance(alloc, mybir.MemoryLocationSet):
            continue
        name = alloc.memorylocations[0].name
        if alloc.kind == "ExternalInput":
            if name != partition_name:
                in_names.append(name)
        elif alloc.kind == "ExternalOutput":
            out_names.append(name)
            shape = tuple(alloc.tensor_shape)
            dtype = mybir.dt.np(alloc.dtype)
            out_avals.append(jax.core.ShapedArray(shape, dtype))
            zero_outs.append(np.zeros(shape, dtype))
    n_params = len(in_names)
    all_in = list(in_names) + out_names
    if partition_name is not None:
        all_in.append(partition_name)

    def _body(*args):
        operands = list(args)
        if partition_name is not None:
            operands.append(partition_id_tensor())
        return tuple(_bass_exec_p.bind(
            *operands, out_avals=tuple(out_avals), in_names=tuple(all_in),
            out_names=tuple(out_names), lowering_input_output_aliases=(),
            sim_require_finite=True, sim_require_nnan=True, nc=nc))

    devices = jax.devices()[:NCORES]
    mesh = Mesh(np.asarray(devices), ("core",))
    n_outs = len(out_avals)
    donate = tuple(range(n_params, n_params + n_outs))
    jitted = jax.jit(
        shard_map(_body, mesh=mesh,
                  in_specs=(PartitionSpec("core"),) * (n_params + n_outs),
                  out_specs=(PartitionSpec("core"),) * n_outs,
                  check_rep=False),
        donate_argnums=donate, keep_unused=True)

    concat_in = [np.concatenate([np.asarray(in_maps[c][nm])
                                 for c in range(NCORES)], axis=0)
                 for nm in in_names]
    concat_zeros = [np.zeros((NCORES * z.shape[0], *z.shape[1:]), z.dtype)
                    for z in zero_outs]
    compiled = jitted.lower(*concat_in, *concat_zeros).compile()

    sharding = NamedSharding(mesh, PartitionSpec("core"))
    dev_in = [jax.device_put(a, sharding) for a in concat_in]
    jax.block_until_ready(dev_in)

    def fresh_zeros():
        zs = [jax.device_put(z, sharding) for z in concat_zeros]
        jax.block_until_ready(zs)
        return zs

    outs = compiled(*dev_in, *fresh_zeros())
    jax.block_until_ready(outs)

    zs = fresh_zeros()
    t0 = time.time()
    outs = compiled(*dev_in, *zs)
    jax.block_until_ready(outs)
    single_s = time.time() - t0

    per_run_s = None
    if n_timed > 0:
        zss = [fresh_zeros() for _ in range(n_timed)]
        t0 = time.time()
        all_outs = [compiled(*dev_in, *zss[i]) for i in range(n_timed)]
        jax.block_until_ready(all_outs)
        per_run_s = (time.time() - t0) / n_timed
        outs = all_outs[-1]

    results = []
    for c in range(NCORES):
        res = {}
        for i, nm in enumerate(out_names):
            a = np.asarray(outs[i])
            res[nm] = a.reshape(NCORES, *out_avals[i].shape)[c]
        results.append(res)
    return results, single_s, per_run_s


# ---------------- host entry point ----------------

def _prepare(inputs):
    import ml_dtypes

    bfloat16 = ml_dtypes.bfloat16
    x = np.asarray(inputs["x"], dtype=np.float32)
    ei = np.asarray(inputs["edge_index"], dtype=np.int64)
    ep = np.asarray(inputs["edge_pairs"], dtype=np.int64)
    W1 = np.asarray(inputs["W1"], dtype=np.float32)
    b1 = np.asarray(inputs["b1"], dtype=np.float32)
    W2 = np.asarray(inputs["W2"], dtype=np.float32)
    b2 = np.asarray(inputs["b2"], dtype=np.float32)
    Wm1 = np.asarray(inputs["Wm1"], dtype=np.float32)
    bm1 = np.asarray(inputs["bm1"], dtype=np.float32)
    Wm2 = np.asarray(inputs["Wm2"], dtype=np.float32)
    bm2 = np.asarray(inputs["bm2"], dtype=np.float32)

    n = N_NODES
    loop = np.arange(n, dtype=np.int64)
    row = np.concatenate([ei[0], loop])
    col = np.concatenate([ei[1], loop])
    deg = np.bincount(col, minlength=n).astype(np.float32)
    dinv = np.where(deg > 0, 1.0 / np.sqrt(np.maximum(deg, 1.0)),
                    0.0).astype(np.float32)

    gidx, sidx, dmsg, rounds, tot = _build_message_streams(row, col, dinv)
    psrc, pdst, omap, group_plan, totp = _build_pair_streams(ep)

    xp_full = (x * dinv[:, None]).astype(bfloat16)
    in_maps = []
    for k in range(NCORES):
        xsl = np.zeros((ROWS, C), bfloat16)
        xsl[:SLICE] = xp_full[k * SLICE:(k + 1) * SLICE]
        dv = np.zeros(ROWS, np.float32)
        dv[:SLICE] = dinv[k * SLICE:(k + 1) * SLICE]
        in_maps.append({
            "xl": xsl,
            "dinv_t": np.ascontiguousarray(dv.reshape(NTILE, 128).T),
            "ident": np.eye(C, dtype=bfloat16),
            "g16": _wrap16(gidx[k]),
            "s16": _wrap16(sidx[k]),
            "dmsg": np.ascontiguousarray(
                dmsg[k].reshape(-1, 128).T).astype(bfloat16),
            "p16s": _wrap16(psrc[k]),
            "p16d": _wrap16(pdst[k]),
            "w1": W1.astype(bfloat16), "w2": W2.astype(bfloat16),
            "wm1a": Wm1[:C].astype(bfloat16),
            "wm1b": Wm1[C:].astype(bfloat16),
            "wm2": Wm2.reshape(C, 1).astype(bfloat16),
            "b1r": b1.reshape(1, C), "b2r": b2.reshape(1, C),
            "bm1c": bm1.reshape(C, 1), "bm2c": bm2.reshape(1, 1),
        })
    return in_maps, rounds, tot, group_plan, totp, omap


def kernel(**inputs):
    import concourse.bacc as bacc

    in_maps, rounds, tot, group_plan, totp, omap = _prepare(inputs)

    nc = bacc.Bacc(None, num_swdge_queues=4)
    nc.m.attributes = (nc.m.attributes or {}) | {"num_swdge_queues": 4}
    _build(nc, rounds, tot, group_plan, totp)
    nc.finalize()

    results, single_s, per_run_s = _run_spmd(nc, in_maps)
    global LAST_RUN_S, LAST_SINGLE_S
    LAST_SINGLE_S = single_s
    LAST_RUN_S = per_run_s if per_run_s is not None else single_s

    pp = N_PAIRS // NCORES
    outv = np.zeros(N_PAIRS, np.float32)
    for k in range(NCORES):
        m = omap[k] >= 0
        outv[k * pp + omap[k][m]] = results[k]["out"][m]
    return outv
